# revision 74
# baseline (speedup 1.0000x reference)
"""Trainium2 Bass kernel for nn_Graph_CNN_Feat_Mesh (Chebyshev GNN decoder).

Strategy (per-core, data-parallel over batch B=256 -> 32/core):
  - All spmms are dense matmuls on the tensor engine (PE) in bf16:
      y = A + L @ (B + L @ (2C)),  A/B/C = feature-space linears of the input.
    L is densified on host; for up4-preceded layers the replication is folded
    into LU = L @ U (contracting the small pre-upsample vertex space).
  - B and A linear terms accumulate directly into the spmm PSUM.
  - Activations live in packed F-layout [(j,Fin) partitions, (b//G)*Vsp + v]
    between layers; the per-layer linear emits V-layout directly; one PE
    transpose per layer returns to F-layout.
  - BatchNorm (training mode, global batch stats) is exact: per-core partial
    sums are AllGather'd across the 8 cores in-kernel (cheaper than
    AllReduce) and summed locally with a K=8 ones-matmul; stats are taken
    per transpose-group so they finish with the last transpose; scale+relu
    is chunked so the next layer starts on early chunks.
  - Weights are host-pre-tiled into [128, *] monoliths and streamed with a
    handful of large DMAs on the gpsimd queue (25ns issue) in consumption
    order; the FC head runs in bf16 with fp32 PSUM.
"""

import numpy as np

B = 256
NCORES = 8
BL = B // NCORES  # 32
EPS = 1e-5
USE_RDMA = False  # remote-DMA BN exchange: unsupported by the timing sim

_CACHE = {}


def _split_W(W):
    W = np.asarray(W, np.float32)
    return W[:, 0::3], W[:, 1::3], W[:, 2::3]


def _dense_L(rows, cols, vals, V):
    L = np.zeros((V, V), np.float32)
    np.add.at(L, (np.asarray(rows), np.asarray(cols)), np.asarray(vals, np.float32))
    return L


def _tile128(a):
    """[S*128, N] -> [128, S*N] block-column layout (pad rows to mult of 128)."""
    a = np.asarray(a)
    S = (a.shape[0] + 127) // 128
    if a.shape[0] != S * 128:
        a = np.concatenate(
            [a, np.zeros((S * 128 - a.shape[0], a.shape[1]), a.dtype)], 0)
    return np.ascontiguousarray(
        a.reshape(S, 128, a.shape[1]).transpose(1, 0, 2).reshape(128, -1))


class _LCfg:
    def __init__(self, name, Vsp, V, Fin, Fout, up4, bn):
        self.name = name
        self.Vsp = Vsp      # source vertex space of C-linear (pre-up4)
        self.V = V          # output vertex count
        self.Fin = Fin
        self.Fout = Fout
        self.G = 128 // Fin          # batches packed on partitions at input
        self.nG = BL // self.G
        self.GF = self.G * Fout      # N of one B/C/A-linear matmul
        self.Gp = 128 // Fout if Fout in (32, 64) else None
        self.nGp = BL // self.Gp if self.Gp else None
        self.up4 = up4
        self.bn = bn
        self.nVt = (V + 127) // 128
        self.nVsp = (Vsp + 127) // 128
        self.BF = BL * Fout          # free width of V-layout per vtile

    def vts(self, t):
        return min(128, self.V - t * 128)

    def sps(self, s):
        return min(128, self.Vsp - s * 128)


CFGS = [
    _LCfg("c0", 80, 320, 64, 64, True, True),
    _LCfg("c1", 320, 320, 64, 32, False, True),
    _LCfg("c2", 320, 1280, 32, 32, True, True),
    _LCfg("c3", 1280, 1280, 32, 3, False, False),
]


def _wbd(W, G, Fin, Fout, which):
    """Block-diagonal rhs weight [128, G*Fout] for the fused linear.
    which: 'A' -> W0 - W2, 'B' -> W1, 'C' -> 2*W2.  col = j*Fout + c."""
    W0, W1, W2 = _split_W(W)
    M = {"A": W0 - W2, "B": W1, "C": 2.0 * W2}[which]  # [Fout, Fin]
    out = np.zeros((128, G * Fout), np.float32)
    for j in range(G):
        out[j * Fin:(j + 1) * Fin, j * Fout:(j + 1) * Fout] = M.T
    return out


# column offsets inside the packed weight blobs
_WOFF = {}
_off = 0
for _cfg in CFGS:
    for _w in "ABC":
        _WOFF[f"{_w}{_cfg.name}"] = (_off, _cfg.GF)
        _off += _cfg.GF
WPACK_N = _off  # bf16 pack cols

# f32 pack: fc1b | per-layer FD-scaled sel blocks (BN partial-sum reduce)
F32_FC1B = 0
F32_SEL = [4, 68, 100]   # selFD for bn layers 0,1,2 (widths 64,32,32)
F32PACK_N = 132
_BN_F = [64, 32, 32]
_BN_FD = [16 * 320, 8 * 320, 8 * 1280]
_BN_NG = [256 * 320, 256 * 320, 256 * 1280]
# with equal per-core/per-group counts, global mu = sum of partition means
# scaled by FD/NG; same factor turns summed (mean^2+var) into E[y^2]
_BN_SCL = [fd / ng for fd, ng in zip(_BN_FD, _BN_NG)]


def _build_host(inputs):
    import ml_dtypes
    bf = ml_dtypes.bfloat16
    f32 = np.float32
    d = {}
    d["xT"] = np.ascontiguousarray(np.asarray(inputs["x"], f32).T).astype(bf)
    d["fc1wt"] = _tile128(
        np.asarray(inputs["fc1_w"], f32).T).astype(bf)           # [128, 16*512]
    d["fc1b"] = np.ascontiguousarray(
        np.asarray(inputs["fc1_b"], f32).reshape(4, 128).T)      # [128,4]
    # fc2: chunk-major (mc), then k-tile: [128, 16*1280]
    w2 = np.asarray(inputs["fc2_w"], f32).T                      # [512, 5120]
    blk = [w2[kt * 128:(kt + 1) * 128, mc * 1280:(mc + 1) * 1280]
           for mc in range(4) for kt in range(4)]
    d["fc2wt"] = np.ascontiguousarray(np.concatenate(blk, 1)).astype(bf)

    L1 = _dense_L(inputs["L1_rows"], inputs["L1_cols"], inputs["L1_vals"], 320)
    L2 = _dense_L(inputs["L2_rows"], inputs["L2_cols"], inputs["L2_vals"], 1280)
    U1 = np.repeat(np.eye(80, dtype=f32), 4, axis=0)    # [320, 80]
    U2 = np.repeat(np.eye(320, dtype=f32), 4, axis=0)   # [1280, 320]
    f8 = ml_dtypes.float8_e4m3
    d["LU0"] = _tile128((L1 @ U1).T).astype(bf)         # [128, 320]
    d["LT1"] = _tile128(L1.T).astype(bf)                # [128, 3*320]
    d["LU2"] = _tile128((L2 @ U2).T).astype(f8)         # [128, 3*1280] fp8
    d["LT2"] = _tile128(L2.T).astype(f8)                # [128, 10*1280] fp8

    Wn = {"c0": "cl0_w", "c1": "cl1_w", "c2": "cl2_w", "c3": "cl3_w"}
    wall = np.zeros((128, WPACK_N), f32)
    for cfg in CFGS:
        W = np.asarray(inputs[Wn[cfg.name]], f32)
        for which in "ABC":
            o, n = _WOFF[f"{which}{cfg.name}"]
            wall[:, o:o + n] = _wbd(W, cfg.G, cfg.Fin, cfg.Fout, which)
    d["wall"] = wall.astype(bf)
    # b3 tiled over the (b, c) column layout of the last-layer PSUM: col = b*3+c
    d["b3row"] = np.ascontiguousarray(
        np.tile(np.asarray(inputs["cl3_b"], f32), BL)[None, :]).astype(bf)

    gbc = np.zeros((128, 6), f32)
    for li, (g, b) in enumerate([("bn0_g", "bn0_b"), ("bn1_g", "bn1_b"),
                                 ("bn2_g", "bn2_b")]):
        F = _BN_F[li]
        gbc[0:F, 2 * li] = np.asarray(inputs[g], f32)
        gbc[0:F, 2 * li + 1] = np.asarray(inputs[b], f32)
    d["gbcol"] = gbc

    f32p = np.zeros((128, F32PACK_N), f32)
    for li in range(3):
        F, o = _BN_F[li], F32_SEL[li]
        v = _BN_SCL[li] if USE_RDMA else _BN_FD[li]
        for j in range(128 // F):
            f32p[j * F:(j + 1) * F, o:o + F] += v * np.eye(F, dtype=f32)
    f32p[:, F32_FC1B:F32_FC1B + 4] = d.pop("fc1b")
    d["f32pack"] = f32p
    # selT_s/selT_t [2F rows, 128]: stc[p,:] = (s[p%F], t[p%F]) via 2 matmuls
    stp = np.zeros((128, 2 * 128), f32)
    for F, ro in [(64, 0), (32, 0)]:
        pass
    sT = np.zeros((128, 256), f32)   # rows k (2F<=128), cols: [0:128]=s-map, [128:256]=t-map
    # build per-F maps stacked by row-offset: F=64 uses rows 0:128, F=32 uses rows 0:64
    sT64 = np.zeros((128, 256), f32)
    sT32 = np.zeros((128, 256), f32)
    for p in range(128):
        sT64[p % 64, p] = 1.0
        sT64[p % 64, 128 + p] = -1.0
        sT32[p % 32, p] = 1.0
        sT32[p % 32, 128 + p] = -1.0
    d["selT64"] = sT64
    d["selT32"] = sT32
    return d


def _build_nc():
    import sys
    for p in ("/opt/trn_rl_repo", "/opt/trn_rl_repo/concourse"):
        if p not in sys.path:
            sys.path.insert(0, p)
    import concourse.bass as bass  # noqa
    import concourse.mybir as mybir
    import concourse.tile as tile
    from concourse import bacc
    from concourse.masks import make_identity

    f32 = mybir.dt.float32
    bf16 = mybir.dt.bfloat16
    fp8 = mybir.dt.float8e4
    DR = mybir.MatmulPerfMode.DoubleRow
    AF = mybir.ActivationFunctionType
    ALU = mybir.AluOpType

    nc = bacc.Bacc(None, target_bir_lowering=False)

    xT = nc.dram_tensor("xT", [2048, BL], bf16, kind="ExternalInput")
    fc1wt = nc.dram_tensor("fc1wt", [128, 16 * 512], bf16, kind="ExternalInput")
    fc2wt = nc.dram_tensor("fc2wt", [128, 16 * 1280], bf16, kind="ExternalInput")
    LU0 = nc.dram_tensor("LU0", [128, 320], bf16, kind="ExternalInput")
    LT1 = nc.dram_tensor("LT1", [128, 3 * 320], bf16, kind="ExternalInput")
    LU2 = nc.dram_tensor("LU2", [128, 3 * 1280], fp8, kind="ExternalInput")
    LT2 = nc.dram_tensor("LT2", [128, 10 * 1280], fp8, kind="ExternalInput")
    wall = nc.dram_tensor("wall", [128, WPACK_N], bf16, kind="ExternalInput")
    gbcol = nc.dram_tensor("gbcol", [128, 6], f32, kind="ExternalInput")
    f32pack = nc.dram_tensor("f32pack", [128, F32PACK_N], f32, kind="ExternalInput")
    selT64 = nc.dram_tensor("selT64", [128, 256], f32, kind="ExternalInput")
    selT32 = nc.dram_tensor("selT32", [128, 256], f32, kind="ExternalInput")
    b3row = nc.dram_tensor("b3row", [1, 96], bf16, kind="ExternalInput")
    ydram = nc.dram_tensor("y", [128, 960], bf16, kind="ExternalOutput")

    with tile.TileContext(nc) as tc:
        with (
            tc.tile_pool(name="const", bufs=1) as constp,
            tc.tile_pool(name="wpool", bufs=1) as wpool,
            tc.tile_pool(name="poolA", bufs=2) as poolA,
            tc.tile_pool(name="poolB", bufs=2) as poolB,
            tc.tile_pool(name="poolC", bufs=1) as poolC,
            tc.tile_pool(name="misc", bufs=1) as miscp,
            tc.tile_pool(name="outp", bufs=3) as outp,
            tc.tile_pool(name="pslin", bufs=2, space="PSUM") as pslin,
            tc.tile_pool(name="psbig", bufs=4, space="PSUM") as psbig,
            tc.tile_pool(name="pstr", bufs=2, space="PSUM") as pstr,
            tc.tile_pool(name="dram", bufs=1, space="DRAM") as dramp,
        ):
            # ---- fc1 inputs first: these DMAs gate the first matmul ----
            fc1w_sb = poolA.tile([128, 16 * 512], bf16, tag="A")
            nc.gpsimd.dma_start(fc1w_sb[:, :4 * 512], fc1wt[:, :4 * 512])
            xT_sb = miscp.tile([128, 16 * BL], bf16, tag="xT")
            nc.gpsimd.dma_start(
                xT_sb[:].rearrange("p (k b) -> p k b", b=BL),
                xT[:].rearrange("(k p) b -> p k b", p=128))
            for kc in range(1, 4):
                nc.gpsimd.dma_start(
                    fc1w_sb[:, kc * 4 * 512:(kc + 1) * 4 * 512],
                    fc1wt[:, kc * 4 * 512:(kc + 1) * 4 * 512])
            f32_sb = constp.tile([128, F32PACK_N], f32, tag="f32pack")
            nc.sync.dma_start(f32_sb[:], f32pack[:])
            selfd_sb = [f32_sb[:, F32_SEL[li]:F32_SEL[li] + _BN_F[li]]
                        for li in range(3)]
            fc1b_sb = f32_sb[:, F32_FC1B:F32_FC1B + 4]

            # ---- small constants (no DMA) ----
            ident_b = constp.tile([128, 128], bf16, tag="identb")
            make_identity(nc, ident_b[:])
            ident_f = constp.tile([1, 1], f32, tag="identf")
            nc.gpsimd.memset(ident_f[:], 1.0)
            eps_t = constp.tile([1, 1], f32, tag="eps")
            nc.gpsimd.memset(eps_t[:], EPS)
            onesn = constp.tile([8, 3], f32, tag="onesn")
            for li in range(3):
                nc.gpsimd.memset(onesn[:, li:li + 1], 1.0 / _BN_NG[li])
            onesv = constp.tile([1, 128], bf16, tag="onesv")
            nc.gpsimd.memset(onesv[:], 1.0)
            sq_warm = constp.tile([1, 1], f32, tag="sqwarm")
            nc.scalar.activation(sq_warm[:], eps_t[:], AF.Sqrt, bias=eps_t[:])

            # ================= FC head (bf16, fp32 psum) =================
            # kt-outer so matmuls start as soon as the first fc1w chunk lands
            h1T = miscp.tile([128, 4 * BL], bf16, tag="h1T")
            ps1 = pslin.tile([128, 4 * BL], f32, tag="lin")
            for mt in range(4):
                for kt in range(16):
                    nc.tensor.matmul(
                        ps1[:, mt * BL:(mt + 1) * BL],
                        fc1w_sb[:, kt * 512 + mt * 128: kt * 512 + (mt + 1) * 128],
                        xT_sb[:, kt * BL:(kt + 1) * BL],
                        start=(kt == 0), stop=(kt == 15))
                nc.scalar.activation(
                    h1T[:, mt * BL:(mt + 1) * BL], ps1[:, mt * BL:(mt + 1) * BL],
                    AF.Relu, bias=fc1b_sb[:, mt:mt + 1])

            # ---- mid-priority loads (small; needed for c0/c1 + BN) ----
            gbc_sb = constp.tile([128, 6], f32, tag="gbc")
            nc.sync.dma_start(gbc_sb[:], gbcol[:])
            epsc = constp.tile([128, 1], f32, tag="epsc")
            nc.gpsimd.memset(epsc[:], EPS)
            sT_sb = {64: constp.tile([128, 256], f32, tag="sT64", name="sT64sb"),
                     32: constp.tile([128, 256], f32, tag="sT32", name="sT32sb")}
            nc.sync.dma_start(sT_sb[64][:], selT64[:])
            nc.sync.dma_start(sT_sb[32][:], selT32[:])
            b3_sb = constp.tile([1, 96], bf16, tag="b3row")
            nc.sync.dma_start(b3_sb[:], b3row[:])

            LUT, LTd = {}, {}
            t = wpool.tile([128, 320], bf16, tag="LU0")
            nc.sync.dma_start(t[:], LU0[:])
            LUT["c0"] = t
            t = wpool.tile([128, 3 * 320], bf16, tag="LT1")
            nc.sync.dma_start(t[:], LT1[:])
            LTd["c0"] = LTd["c1"] = LUT["c1"] = t
            wall_sb = wpool.tile([128, WPACK_N], bf16, tag="wall")
            nc.sync.dma_start(wall_sb[:], wall[:])
            W_sb = {}
            for cfg in CFGS:
                for w in "ABC":
                    o, n = _WOFF[f"{w}{cfg.name}"]
                    W_sb[f"{w}{cfg.name}"] = wall_sb[:, o:o + n]

            # ================= fc2 (streamed in 4 column-chunks) =========
            # psum partition = (v0%2)*64+f, col = mi*BL+b ; channels c = v0*64+f.
            # dest: XF0[(b%2)*64+f, (b//2)*80 + v0],  v0 = 2*(mc*10+mi)+p0
            XF0 = poolC.tile([128, 16 * 80], bf16, tag="XF0")
            cfg0 = CFGS[0]
            XFrep0 = poolA.tile([128, cfg0.nG * cfg0.V], bf16, tag="A")
            s_r0 = XF0[:].rearrange("p (g w) -> p g w", w=80)
            d_r0 = XFrep0[:].rearrange("p (g w r) -> p g w r", w=80, r=4)
            for mc in range(4):
                wch = poolB.tile([128, 4 * 1280], bf16, tag="B")
                nc.gpsimd.dma_start(
                    wch[:], fc2wt[:, mc * 4 * 1280:(mc + 1) * 4 * 1280])
                ps2 = psbig.tile([128, 10 * BL], f32, tag="big")
                for mi in range(10):
                    for kt in range(4):
                        nc.tensor.matmul(
                            ps2[:, mi * BL:(mi + 1) * BL],
                            wch[:, kt * 1280 + mi * 128: kt * 1280 + (mi + 1) * 128],
                            h1T[:, kt * BL:(kt + 1) * BL],
                            start=(kt == 0), stop=(kt == 3))
                src4 = ps2[:].rearrange("p (i g j) -> p i g j", g=16, j=2)
                dst4 = XF0[:].rearrange("p (g u q) -> p g u q", u=40, q=2)
                for p0 in range(2):
                    for j in range(2):
                        nc.scalar.activation(
                            dst4[j * 64:(j + 1) * 64, :,
                                 mc * 10:(mc + 1) * 10, p0]
                            .rearrange("p g i -> p i g"),
                            src4[p0 * 64:(p0 + 1) * 64, :, :, j],
                            AF.Copy)
                # up4-replicate this chunk's w-range (w = v0 in [20mc, 20mc+20))
                for r in range(4):
                    nc.vector.tensor_copy(
                        d_r0[:, :, 20 * mc:20 * (mc + 1), r],
                        s_r0[:, :, 20 * mc:20 * (mc + 1)])

            # ---- big late loads (needed at c2; stream during c0/c1) ----
            t = wpool.tile([128, 3 * 1280], fp8, tag="LU2")
            nc.gpsimd.dma_start(t[:], LU2[:])
            LUT["c2"] = t
            t = wpool.tile([128, 10 * 1280], fp8, tag="LT2")
            nc.gpsimd.dma_start(t[:, :5 * 1280], LT2[:, :5 * 1280])
            nc.gpsimd.dma_start(t[:, 5 * 1280:], LT2[:, 5 * 1280:])
            LTd["c2"] = LTd["c3"] = LUT["c3"] = t

            # ================= cheby layers =================
            if USE_RDMA:
                rsem = nc.alloc_semaphore("bn_rsem")
                lsem = nc.alloc_semaphore("bn_lsem")
                rbufs = [constp.tile([128, 16], f32, tag=f"rbuf{i}",
                                     name=f"rbuf{i}")
                         for i in range(3)]
            XF_cur = XF0
            XFrep_cur = XFrep0
            ar_idx = 0

            for li, cfg in enumerate(CFGS):
                V, Vsp, F = cfg.V, cfg.Vsp, cfg.Fout
                BF = cfg.BF
                last = cfg.name == "c3"
                XFrep = XFrep_cur if cfg.up4 else XF_cur

                fp8sp = cfg.name in ("c2", "c3")
                sp_dt = fp8 if fp8sp else bf16
                # --- C linear (in Vsp space) ---
                XC = poolC.tile([128, cfg.nVsp * BL * F], sp_dt, tag="XC")
                gpack = max(1, 512 // cfg.GF)
                for s in range(cfg.nVsp):
                    ssz = cfg.sps(s)
                    for g0 in range(0, cfg.nG, gpack):
                        gn = min(gpack, cfg.nG - g0)
                        pc = pslin.tile([128, 512], f32, tag="lin")
                        for gi in range(gn):
                            g = g0 + gi
                            nc.tensor.matmul(
                                pc[:ssz, gi * cfg.GF:(gi + 1) * cfg.GF],
                                XF_cur[:, g * Vsp + s * 128:
                                       g * Vsp + s * 128 + ssz],
                                W_sb[f"C{cfg.name}"][:],
                                start=True, stop=True)
                        xdst = XC[:ssz, s * BL * F + g0 * cfg.GF:
                                  s * BL * F + (g0 + gn) * cfg.GF]
                        if (s + g0) % 2 == 0:
                            nc.scalar.activation(
                                xdst, pc[:ssz, :gn * cfg.GF], AF.Copy)
                        else:
                            nc.vector.tensor_copy(
                                xdst, pc[:ssz, :gn * cfg.GF])

                # --- inner = LU @ (2C) + B ;  y = L @ inner + A ---
                Xin = poolB.tile([128, cfg.nVt * BF], sp_dt, tag="B")
                ytile = poolC.tile([128, cfg.nVt * BF], bf16, tag="YT")
                if not last:
                    # transpose/stat state, filled per-tile inside phase 1
                    Gp, nGp = cfg.Gp, cfg.nGp
                    nq = (nGp + 3) // 4
                    XFn = poolA.tile([128, nGp * V], bf16, tag="A")
                    dstv = XFn[:].rearrange("p (g v) -> p g v", v=V)
                    nch = cfg.nVt * nq
                    bnst = miscp.tile([128, nch * 6], f32, tag="bnst")
                    chn = 0
                for phase in range(2):
                    srcL = LUT[cfg.name] if phase == 0 else LTd[cfg.name]
                    nS = cfg.nVsp if phase == 0 else cfg.nVt
                    ssizes = ([cfg.sps(s) for s in range(nS)] if phase == 0
                              else [cfg.vts(s) for s in range(nS)])
                    rhs = XC if phase == 0 else Xin
                    rhs_w = BL * F if phase == 0 else BF
                    Wacc = W_sb[f"B{cfg.name}" if phase == 0 else f"A{cfg.name}"]
                    dst = Xin if phase == 0 else ytile
                    for t in range(cfg.nVt):
                        vsz = cfg.vts(t)
                        for n0 in range(0, BF, 512):
                            n1 = min(n0 + 512, BF)
                            pw = n1 - n0
                            pc0 = n0
                            pi = psbig.tile([128, 512], f32, tag="big")
                            if True:
                                if fp8sp:
                                    # fp8 DoubleRow: contract 2 s-tiles/pass
                                    srcr = srcL[:].rearrange(
                                        "p (s v) -> p s v", v=V)
                                    rhsr = rhs[:].rearrange(
                                        "p (s n) -> p s n", n=rhs_w)
                                    for s0 in range(0, nS, 2):
                                        if s0 + 1 < nS and \
                                                ssizes[s0 + 1] == 128:
                                            nc.tensor.matmul(
                                                pi[:vsz, n0 - pc0:n1 - pc0],
                                                srcr[:, s0:s0 + 2,
                                                     t * 128:t * 128 + vsz],
                                                rhsr[:, s0:s0 + 2, n0:n1],
                                                start=(s0 == 0), stop=False,
                                                skip_group_check=True,
                                                perf_mode=DR)
                                        else:
                                            for s in range(s0, min(s0 + 2,
                                                                   nS)):
                                                ssz = ssizes[s]
                                                nc.tensor.matmul(
                                                    pi[:vsz,
                                                       n0 - pc0:n1 - pc0],
                                                    srcL[:ssz, s * V + t * 128:
                                                         s * V + t * 128 + vsz],
                                                    rhs[:ssz, s * rhs_w + n0:
                                                        s * rhs_w + n1],
                                                    start=(s == 0), stop=False,
                                                    skip_group_check=True)
                                else:
                                    for s in range(nS):
                                        ssz = ssizes[s]
                                        nc.tensor.matmul(
                                            pi[:vsz, n0 - pc0:n1 - pc0],
                                            srcL[:ssz, s * V + t * 128:
                                                 s * V + t * 128 + vsz],
                                            rhs[:ssz, s * rhs_w + n0:
                                                s * rhs_w + n1],
                                            start=(s == 0), stop=False,
                                            skip_group_check=True)
                                for g in range(n0 // cfg.GF,
                                               (n1 + cfg.GF - 1) // cfg.GF):
                                    nc.tensor.matmul(
                                        pi[:vsz, g * cfg.GF - pc0:
                                           (g + 1) * cfg.GF - pc0],
                                        XFrep[:, g * V + t * 128:
                                              g * V + t * 128 + vsz],
                                        Wacc[:],
                                        start=False,
                                        stop=(not (last and phase == 1)),
                                        skip_group_check=True)
                                if last and phase == 1:
                                    # fold cl3 bias: += ones^T @ b3row
                                    nc.tensor.matmul(
                                        pi[:vsz, n0 - pc0:n1 - pc0],
                                        onesv[:1, :vsz],
                                        b3_sb[:1, n0:n1],
                                        start=False, stop=True,
                                        skip_group_check=True)
                            if last and phase == 1:
                                nc.vector.tensor_copy(
                                    dst[:vsz, t * BF + pc0: t * BF + pc0 + pw],
                                    pi[:vsz, :pw])
                            elif phase == 0:
                                if (t * BF + n0) // 512 % 2 == 0:
                                    nc.scalar.activation(
                                        dst[:vsz,
                                            t * BF + pc0: t * BF + pc0 + pw],
                                        pi[:vsz, :pw], AF.Copy)
                                else:
                                    nc.vector.tensor_copy(
                                        dst[:vsz,
                                            t * BF + pc0: t * BF + pc0 + pw],
                                        pi[:vsz, :pw])
                            else:
                                if (t * BF + n0) // 512 % 2 == 0:
                                    nc.vector.tensor_copy(
                                        dst[:vsz,
                                            t * BF + pc0: t * BF + pc0 + pw],
                                        pi[:vsz, :pw])
                                else:
                                    nc.scalar.activation(
                                        dst[:vsz,
                                            t * BF + pc0: t * BF + pc0 + pw],
                                        pi[:vsz, :pw], AF.Copy)
                        if phase == 1 and not last:
                            # back-transpose this tile now so PE overlaps
                            # later tiles' spmm matmuls
                            for qi0 in range(nq):
                                q0 = qi0 * 4
                                qn = min(4, nGp - q0)
                                pt = pstr.tile([128, 512], bf16, tag="tr")
                                for qi in range(qn):
                                    gp = q0 + qi
                                    nc.tensor.transpose(
                                        pt[:, qi * vsz: qi * vsz + vsz],
                                        ytile[:vsz, t * BF + gp * 128:
                                              t * BF + (gp + 1) * 128],
                                        ident_b[:vsz, :vsz])
                                reg = dstv[:, q0:q0 + qn,
                                           t * 128:t * 128 + vsz]
                                nc.scalar.activation(
                                    reg,
                                    pt[:].rearrange("p (q v) -> p q v",
                                                    v=vsz)[:, :qn, :],
                                    AF.Copy)
                                nc.vector.bn_stats(
                                    bnst[:, chn * 6:(chn + 1) * 6],
                                    pt[:, :qn * vsz])
                                chn += 1

                if not last:
                    aggr = miscp.tile([128, 2], f32, tag="aggr")
                    nc.vector.bn_aggr(
                        aggr[:], bnst[:, :chn * 6]
                        .rearrange("p (c s) -> p c s", s=6))
                    part = miscp.tile([128, 2], f32, tag="part")
                    if USE_RDMA and ar_idx > 0:
                        # prior layer's sends must have drained before reuse
                        nc.vector.wait_ge(lsem, 112 * ar_idx)
                    nc.vector.tensor_tensor(
                        out=part[:, 1:2], in0=aggr[:, 0:1], in1=aggr[:, 0:1],
                        op=ALU.mult)
                    nc.vector.tensor_tensor(
                        out=part[:, 1:2], in0=part[:, 1:2], in1=aggr[:, 1:2],
                        op=ALU.add)
                    if USE_RDMA:
                        nc.vector.tensor_copy(part[:, 0:1], aggr[:, 0:1])
                        rb = rbufs[ar_idx]
                        nc.vector.tensor_copy(rb[:, 0:2], part[:])
                        for k in range(1, 8):
                            nc.gpsimd.remote_dma_broadcast(
                                rb[:, 2 * k:2 * k + 2], part[:],
                                remote_sem=rsem, local_sem=lsem,
                                rdests=[(0, k) if i == k else None
                                        for i in range(8)])
                        nc.gpsimd.trigger_dma(count=None)
                        nc.vector.wait_ge(rsem, 14 * (ar_idx + 1))
                        nc.vector.tensor_tensor(
                            out=rb[:, 0:8], in0=rb[:, 0:8], in1=rb[:, 8:16],
                            op=ALU.add)
                        nc.vector.tensor_tensor(
                            out=rb[:, 0:4], in0=rb[:, 0:4], in1=rb[:, 4:8],
                            op=ALU.add)
                        nc.vector.tensor_tensor(
                            out=rb[:, 0:2], in0=rb[:, 0:2], in1=rb[:, 2:4],
                            op=ALU.add)
                        pst = pslin.tile([128, 512], f32, tag="lin")
                        nc.tensor.matmul(pst[:1, :F], rb[:, 0:1],
                                         selfd_sb[li], start=True, stop=True)
                        nc.tensor.matmul(pst[:1, F:2 * F], rb[:, 1:2],
                                         selfd_sb[li], start=True, stop=True)
                        stats_g = miscp.tile([1, 2 * F], f32, tag="statg")
                        nc.vector.tensor_copy(stats_g[:], pst[:1, :2 * F])
                    else:
                        pst = pslin.tile([128, 512], f32, tag="lin")
                        nc.tensor.matmul(pst[:1, :F], aggr[:, 0:1],
                                         selfd_sb[li], start=True, stop=True)
                        nc.tensor.matmul(pst[:1, F:2 * F], part[:, 1:2],
                                         selfd_sb[li], start=True, stop=True)
                        stats_l = miscp.tile([1, 2 * F], f32, tag="statl")
                        nc.vector.tensor_copy(stats_l[:], pst[:1, :2 * F])
                        bin_ = dramp.tile([1, 2 * F], f32, tag=f"arin{ar_idx}")
                        bout = dramp.tile([8, 2 * F], f32, tag=f"arout{ar_idx}")
                        nc.sync.dma_start(bin_[:], stats_l[:])
                        nc.gpsimd.collective_compute(
                            "AllGather", ALU.bypass,
                            replica_groups=[list(range(NCORES))],
                            ins=[bin_.opt()], outs=[bout.opt()])
                        sg8 = miscp.tile([8, 2 * F], f32, tag="sg8")
                        nc.sync.dma_start(sg8[:], bout[:])
                        psg = pslin.tile([128, 512], f32, tag="lin", name="psg")
                        nc.tensor.matmul(psg[:2 * F, 0:1], sg8[:, :2 * F],
                                         onesn[:, li:li + 1],
                                         start=True, stop=True)
                    # column form, all vars at partitions [0:F]
                    # stg col0 rows [0:F]=mu, [F:2F]=E[y^2]
                    stg = miscp.tile([128, 1], f32, tag="stg")
                    nc.vector.tensor_copy(stg[:2 * F, :], psg[:2 * F, 0:1])
                    w = miscp.tile([128, 2], f32, tag="bnw")
                    st = miscp.tile([128, 2], f32, tag="st")
                    # w1 = mu*mu - E[y^2] = -var  (scalar2 reads rows F:2F)
                    nc.vector.tensor_scalar(
                        out=w[0:F, 1:2], in0=stg[0:F, :],
                        scalar1=stg[0:F, :], scalar2=stg[F:2 * F, :],
                        op0=ALU.mult, op1=ALU.subtract)
                    nc.scalar.activation(w[0:F, 1:2], w[0:F, 1:2],
                                         AF.Sqrt, bias=epsc[0:F, :],
                                         scale=-1.0)
                    nc.vector.reciprocal(w[0:F, 1:2], w[0:F, 1:2])
                    nc.vector.tensor_tensor(out=st[0:F, 0:1],
                                            in0=w[0:F, 1:2],
                                            in1=gbc_sb[0:F, 2 * li:2 * li + 1],
                                            op=ALU.mult)
                    # st1 = mu*s - beta = -t  (negated by the t-map below)
                    nc.vector.tensor_scalar(
                        out=st[0:F, 1:2], in0=stg[0:F, :],
                        scalar1=st[0:F, 0:1],
                        scalar2=gbc_sb[0:F, 2 * li + 1:2 * li + 2],
                        op0=ALU.mult, op1=ALU.subtract)
                    # broadcast: col0 via +map, col1 via -map (restores t)
                    psc = pslin.tile([128, 512], f32, tag="lin", name="psc")
                    nc.tensor.matmul(psc[:, 0:1], sT_sb[F][:F, 0:128],
                                     st[:F, 0:1], start=True, stop=True)
                    nc.tensor.matmul(psc[:, 1:2], sT_sb[F][:F, 128:256],
                                     st[:F, 1:2], start=True, stop=True)
                    stc = miscp.tile([128, 2], f32, tag=f"stc{ar_idx}")
                    nc.vector.tensor_copy(stc[:], psc[:, 0:2])
                    ar_idx += 1
                    # chunked scale+relu (+ up4 replication for next layer)
                    ncfg = CFGS[li + 1]
                    if ncfg.up4:
                        XFrep_cur = poolA.tile(
                            [128, ncfg.nG * ncfg.V], bf16, tag="A")
                        s_r = XFn[:].rearrange("p (g w) -> p g w", w=V)
                        d_r = XFrep_cur[:].rearrange(
                            "p (g w r) -> p g w r", w=V, r=4)
                    nap = max(1, nGp // 4)
                    bnds = [0, 1] + list(range(1 + nap, nGp, nap)) + [nGp]
                    bnds = sorted(set(b for b in bnds if b <= nGp))
                    for q0, q1 in zip(bnds[:-1], bnds[1:]):
                        c0_, c1_ = q0 * V, q1 * V
                        cm = c0_ + (c1_ - c0_) * 5 // 9  # Act a bit slower
                        nc.scalar.activation(
                            XFn[:, c0_:cm], XFn[:, c0_:cm],
                            AF.Relu, scale=stc[:, 0:1], bias=stc[:, 1:2])
                        nc.vector.tensor_scalar(
                            out=XFn[:, cm:c1_], in0=XFn[:, cm:c1_],
                            scalar1=stc[:, 0:1], scalar2=stc[:, 1:2],
                            op0=ALU.mult, op1=ALU.add)
                        nc.vector.tensor_scalar_max(
                            XFn[:, cm:c1_], XFn[:, cm:c1_], 0.0)
                        if ncfg.up4:
                            for r in range(4):
                                nc.vector.tensor_copy(
                                    d_r[:, q0:q1, :, r], s_r[:, q0:q1, :])
                    XF_cur = XFn
                else:
                    # --- output: ship ytile [v-part, (b,c)] as-is; host
                    # untangles the (p, t, b, c) layout in numpy ---
                    for t0, t1 in ((0, 4), (4, 8), (8, 10)):
                        nc.sync.dma_start(
                            ydram[:, t0 * BF:t1 * BF],
                            ytile[:, t0 * BF:t1 * BF])

    nc.compile()
    return nc


def kernel(**inputs):
    import sys
    for p in ("/opt/trn_rl_repo", "/opt/trn_rl_repo/concourse"):
        if p not in sys.path:
            sys.path.insert(0, p)
    from concourse.bass_utils import run_bass_kernel_spmd

    host = _build_host(inputs)

    if "nc" not in _CACHE:
        _CACHE["nc"] = _build_nc()
    nc = _CACHE["nc"]

    in_maps = []
    for c in range(NCORES):
        m = {k: v for k, v in host.items() if k != "xT"}
        m["xT"] = np.ascontiguousarray(host["xT"][:, c * BL:(c + 1) * BL])
        in_maps.append(m)
    res = run_bass_kernel_spmd(nc, in_maps, core_ids=list(range(NCORES)))
    # y[p, t*96 + b*3 + c] -> out[b, t*128+p, c]
    outs = []
    for r in res.results:
        y = np.asarray(r["y"], np.float32).reshape(128, 10, BL, 3)
        outs.append(y.transpose(2, 1, 0, 3).reshape(BL, 1280, 3))
    return np.concatenate(outs, axis=0).astype(np.float32)


if __name__ == "__main__":
    import reference as R
    inp = R.setup_inputs()
    inp = {k: np.asarray(v) for k, v in inp.items()}
    act = kernel(**inp)
    exp = np.asarray(R.reference(**inp))
    err = np.linalg.norm(act - exp) / np.linalg.norm(exp)
    print("Relative error:", err)


# revision 75
# speedup vs baseline: 1.0081x; 1.0081x over previous
"""Trainium2 Bass kernel for nn_Graph_CNN_Feat_Mesh (Chebyshev GNN decoder).

Strategy (per-core, data-parallel over batch B=256 -> 32/core):
  - All spmms are dense matmuls on the tensor engine (PE) in bf16:
      y = A + L @ (B + L @ (2C)),  A/B/C = feature-space linears of the input.
    L is densified on host; for up4-preceded layers the replication is folded
    into LU = L @ U (contracting the small pre-upsample vertex space).
  - B and A linear terms accumulate directly into the spmm PSUM.
  - Activations live in packed F-layout [(j,Fin) partitions, (b//G)*Vsp + v]
    between layers; the per-layer linear emits V-layout directly; one PE
    transpose per layer returns to F-layout.
  - BatchNorm (training mode, global batch stats) is exact: per-core partial
    sums are AllGather'd across the 8 cores in-kernel (cheaper than
    AllReduce) and summed locally with a K=8 ones-matmul; stats are taken
    per transpose-group so they finish with the last transpose; scale+relu
    is chunked so the next layer starts on early chunks.
  - Weights are host-pre-tiled into [128, *] monoliths and streamed with a
    handful of large DMAs on the gpsimd queue (25ns issue) in consumption
    order; the FC head runs in bf16 with fp32 PSUM.
"""

import numpy as np

B = 256
NCORES = 8
BL = B // NCORES  # 32
EPS = 1e-5
USE_RDMA = False  # remote-DMA BN exchange: unsupported by the timing sim

_CACHE = {}


def _split_W(W):
    W = np.asarray(W, np.float32)
    return W[:, 0::3], W[:, 1::3], W[:, 2::3]


def _dense_L(rows, cols, vals, V):
    L = np.zeros((V, V), np.float32)
    np.add.at(L, (np.asarray(rows), np.asarray(cols)), np.asarray(vals, np.float32))
    return L


def _tile128(a):
    """[S*128, N] -> [128, S*N] block-column layout (pad rows to mult of 128)."""
    a = np.asarray(a)
    S = (a.shape[0] + 127) // 128
    if a.shape[0] != S * 128:
        a = np.concatenate(
            [a, np.zeros((S * 128 - a.shape[0], a.shape[1]), a.dtype)], 0)
    return np.ascontiguousarray(
        a.reshape(S, 128, a.shape[1]).transpose(1, 0, 2).reshape(128, -1))


class _LCfg:
    def __init__(self, name, Vsp, V, Fin, Fout, up4, bn):
        self.name = name
        self.Vsp = Vsp      # source vertex space of C-linear (pre-up4)
        self.V = V          # output vertex count
        self.Fin = Fin
        self.Fout = Fout
        self.G = 128 // Fin          # batches packed on partitions at input
        self.nG = BL // self.G
        self.GF = self.G * Fout      # N of one B/C/A-linear matmul
        self.Gp = 128 // Fout if Fout in (32, 64) else None
        self.nGp = BL // self.Gp if self.Gp else None
        self.up4 = up4
        self.bn = bn
        self.nVt = (V + 127) // 128
        self.nVsp = (Vsp + 127) // 128
        self.BF = BL * Fout          # free width of V-layout per vtile

    def vts(self, t):
        return min(128, self.V - t * 128)

    def sps(self, s):
        return min(128, self.Vsp - s * 128)


CFGS = [
    _LCfg("c0", 80, 320, 64, 64, True, True),
    _LCfg("c1", 320, 320, 64, 32, False, True),
    _LCfg("c2", 320, 1280, 32, 32, True, True),
    _LCfg("c3", 1280, 1280, 32, 3, False, False),
]


def _wbd(W, G, Fin, Fout, which):
    """Block-diagonal rhs weight [128, G*Fout] for the fused linear.
    which: 'A' -> W0 - W2, 'B' -> W1, 'C' -> 2*W2.  col = j*Fout + c."""
    W0, W1, W2 = _split_W(W)
    M = {"A": W0 - W2, "B": W1, "C": 2.0 * W2}[which]  # [Fout, Fin]
    out = np.zeros((128, G * Fout), np.float32)
    for j in range(G):
        out[j * Fin:(j + 1) * Fin, j * Fout:(j + 1) * Fout] = M.T
    return out


# column offsets inside the packed weight blobs
_WOFF = {}
_off = 0
for _cfg in CFGS:
    for _w in "ABC":
        _WOFF[f"{_w}{_cfg.name}"] = (_off, _cfg.GF)
        _off += _cfg.GF
WPACK_N = _off  # bf16 pack cols

# f32 pack: fc1b | per-layer FD-scaled sel blocks (BN partial-sum reduce)
F32_FC1B = 0
F32_SEL = [4, 68, 100]   # selFD for bn layers 0,1,2 (widths 64,32,32)
F32PACK_N = 132
_BN_F = [64, 32, 32]
_BN_FD = [16 * 320, 8 * 320, 8 * 1280]
_BN_NG = [256 * 320, 256 * 320, 256 * 1280]
# with equal per-core/per-group counts, global mu = sum of partition means
# scaled by FD/NG; same factor turns summed (mean^2+var) into E[y^2]
_BN_SCL = [fd / ng for fd, ng in zip(_BN_FD, _BN_NG)]


def _build_host(inputs):
    import ml_dtypes
    bf = ml_dtypes.bfloat16
    f32 = np.float32
    d = {}
    d["xT"] = np.ascontiguousarray(np.asarray(inputs["x"], f32).T).astype(bf)
    d["fc1wt"] = _tile128(
        np.asarray(inputs["fc1_w"], f32).T).astype(bf)           # [128, 16*512]
    d["fc1b"] = np.ascontiguousarray(
        np.asarray(inputs["fc1_b"], f32).reshape(4, 128).T)      # [128,4]
    # fc2: chunk-major (mc), then k-tile: [128, 16*1280]
    w2 = np.asarray(inputs["fc2_w"], f32).T                      # [512, 5120]
    blk = [w2[kt * 128:(kt + 1) * 128, mc * 1280:(mc + 1) * 1280]
           for mc in range(4) for kt in range(4)]
    d["fc2wt"] = np.ascontiguousarray(np.concatenate(blk, 1)).astype(bf)

    L1 = _dense_L(inputs["L1_rows"], inputs["L1_cols"], inputs["L1_vals"], 320)
    L2 = _dense_L(inputs["L2_rows"], inputs["L2_cols"], inputs["L2_vals"], 1280)
    U1 = np.repeat(np.eye(80, dtype=f32), 4, axis=0)    # [320, 80]
    U2 = np.repeat(np.eye(320, dtype=f32), 4, axis=0)   # [1280, 320]
    f8 = ml_dtypes.float8_e4m3
    d["LU0"] = _tile128((L1 @ U1).T).astype(bf)         # [128, 320]
    d["LT1"] = _tile128(L1.T).astype(bf)                # [128, 3*320]
    d["LU2"] = _tile128((L2 @ U2).T).astype(f8)         # [128, 3*1280] fp8
    d["LT2"] = _tile128(L2.T).astype(f8)                # [128, 10*1280] fp8

    Wn = {"c0": "cl0_w", "c1": "cl1_w", "c2": "cl2_w", "c3": "cl3_w"}
    wall = np.zeros((128, WPACK_N), f32)
    for cfg in CFGS:
        W = np.asarray(inputs[Wn[cfg.name]], f32)
        for which in "ABC":
            o, n = _WOFF[f"{which}{cfg.name}"]
            wall[:, o:o + n] = _wbd(W, cfg.G, cfg.Fin, cfg.Fout, which)
    d["wall"] = wall.astype(bf)
    # b3 tiled over the (b, c) column layout of the last-layer PSUM: col = b*3+c
    d["b3row"] = np.ascontiguousarray(
        np.tile(np.asarray(inputs["cl3_b"], f32), BL)[None, :]).astype(bf)

    gbc = np.zeros((128, 6), f32)
    for li, (g, b) in enumerate([("bn0_g", "bn0_b"), ("bn1_g", "bn1_b"),
                                 ("bn2_g", "bn2_b")]):
        F = _BN_F[li]
        gbc[0:F, 2 * li] = np.asarray(inputs[g], f32)
        gbc[0:F, 2 * li + 1] = np.asarray(inputs[b], f32)
    d["gbcol"] = gbc

    f32p = np.zeros((128, F32PACK_N), f32)
    for li in range(3):
        F, o = _BN_F[li], F32_SEL[li]
        v = _BN_SCL[li] if USE_RDMA else _BN_FD[li]
        for j in range(128 // F):
            f32p[j * F:(j + 1) * F, o:o + F] += v * np.eye(F, dtype=f32)
    f32p[:, F32_FC1B:F32_FC1B + 4] = d.pop("fc1b")
    d["f32pack"] = f32p
    # selT_s/selT_t [2F rows, 128]: stc[p,:] = (s[p%F], t[p%F]) via 2 matmuls
    stp = np.zeros((128, 2 * 128), f32)
    for F, ro in [(64, 0), (32, 0)]:
        pass
    sT = np.zeros((128, 256), f32)   # rows k (2F<=128), cols: [0:128]=s-map, [128:256]=t-map
    # build per-F maps stacked by row-offset: F=64 uses rows 0:128, F=32 uses rows 0:64
    sT64 = np.zeros((128, 256), f32)
    sT32 = np.zeros((128, 256), f32)
    for p in range(128):
        sT64[p % 64, p] = 1.0
        sT64[p % 64, 128 + p] = -1.0
        sT32[p % 32, p] = 1.0
        sT32[p % 32, 128 + p] = -1.0
    d["selT64"] = sT64
    d["selT32"] = sT32
    return d


def _build_nc():
    import sys
    for p in ("/opt/trn_rl_repo", "/opt/trn_rl_repo/concourse"):
        if p not in sys.path:
            sys.path.insert(0, p)
    import concourse.bass as bass  # noqa
    import concourse.mybir as mybir
    import concourse.tile as tile
    from concourse import bacc
    from concourse.masks import make_identity

    f32 = mybir.dt.float32
    bf16 = mybir.dt.bfloat16
    fp8 = mybir.dt.float8e4
    DR = mybir.MatmulPerfMode.DoubleRow
    AF = mybir.ActivationFunctionType
    ALU = mybir.AluOpType

    nc = bacc.Bacc(None, target_bir_lowering=False)

    xT = nc.dram_tensor("xT", [2048, BL], bf16, kind="ExternalInput")
    fc1wt = nc.dram_tensor("fc1wt", [128, 16 * 512], bf16, kind="ExternalInput")
    fc2wt = nc.dram_tensor("fc2wt", [128, 16 * 1280], bf16, kind="ExternalInput")
    LU0 = nc.dram_tensor("LU0", [128, 320], bf16, kind="ExternalInput")
    LT1 = nc.dram_tensor("LT1", [128, 3 * 320], bf16, kind="ExternalInput")
    LU2 = nc.dram_tensor("LU2", [128, 3 * 1280], fp8, kind="ExternalInput")
    LT2 = nc.dram_tensor("LT2", [128, 10 * 1280], fp8, kind="ExternalInput")
    wall = nc.dram_tensor("wall", [128, WPACK_N], bf16, kind="ExternalInput")
    gbcol = nc.dram_tensor("gbcol", [128, 6], f32, kind="ExternalInput")
    f32pack = nc.dram_tensor("f32pack", [128, F32PACK_N], f32, kind="ExternalInput")
    selT64 = nc.dram_tensor("selT64", [128, 256], f32, kind="ExternalInput")
    selT32 = nc.dram_tensor("selT32", [128, 256], f32, kind="ExternalInput")
    b3row = nc.dram_tensor("b3row", [1, 96], bf16, kind="ExternalInput")
    ydram = nc.dram_tensor("y", [128, 960], bf16, kind="ExternalOutput")

    with tile.TileContext(nc) as tc:
        with (
            tc.tile_pool(name="const", bufs=1) as constp,
            tc.tile_pool(name="wpool", bufs=1) as wpool,
            tc.tile_pool(name="poolA", bufs=2) as poolA,
            tc.tile_pool(name="poolB", bufs=2) as poolB,
            tc.tile_pool(name="poolC", bufs=1) as poolC,
            tc.tile_pool(name="misc", bufs=1) as miscp,
            tc.tile_pool(name="outp", bufs=3) as outp,
            tc.tile_pool(name="pslin", bufs=2, space="PSUM") as pslin,
            tc.tile_pool(name="psbig", bufs=4, space="PSUM") as psbig,
            tc.tile_pool(name="pstr", bufs=2, space="PSUM") as pstr,
            tc.tile_pool(name="dram", bufs=1, space="DRAM") as dramp,
        ):
            # ---- fc1 inputs first: these DMAs gate the first matmul ----
            fc1w_sb = poolA.tile([128, 16 * 512], bf16, tag="A")
            nc.gpsimd.dma_start(fc1w_sb[:, :4 * 512], fc1wt[:, :4 * 512])
            xT_sb = miscp.tile([128, 16 * BL], bf16, tag="xT")
            nc.gpsimd.dma_start(
                xT_sb[:].rearrange("p (k b) -> p k b", b=BL),
                xT[:].rearrange("(k p) b -> p k b", p=128))
            for kc in range(1, 4):
                nc.gpsimd.dma_start(
                    fc1w_sb[:, kc * 4 * 512:(kc + 1) * 4 * 512],
                    fc1wt[:, kc * 4 * 512:(kc + 1) * 4 * 512])
            f32_sb = constp.tile([128, F32PACK_N], f32, tag="f32pack")
            nc.sync.dma_start(f32_sb[:], f32pack[:])
            selfd_sb = [f32_sb[:, F32_SEL[li]:F32_SEL[li] + _BN_F[li]]
                        for li in range(3)]
            fc1b_sb = f32_sb[:, F32_FC1B:F32_FC1B + 4]

            # ---- small constants (no DMA) ----
            ident_b = constp.tile([128, 128], bf16, tag="identb")
            make_identity(nc, ident_b[:])
            ident_f = constp.tile([1, 1], f32, tag="identf")
            nc.gpsimd.memset(ident_f[:], 1.0)
            eps_t = constp.tile([1, 1], f32, tag="eps")
            nc.gpsimd.memset(eps_t[:], EPS)
            onesn = constp.tile([8, 3], f32, tag="onesn")
            for li in range(3):
                nc.gpsimd.memset(onesn[:, li:li + 1], 1.0 / _BN_NG[li])
            onesv = constp.tile([1, 128], bf16, tag="onesv")
            nc.gpsimd.memset(onesv[:], 1.0)
            sq_warm = constp.tile([1, 1], f32, tag="sqwarm")
            nc.scalar.activation(sq_warm[:], eps_t[:], AF.Sqrt, bias=eps_t[:])

            # ================= FC head (bf16, fp32 psum) =================
            # kt-outer so matmuls start as soon as the first fc1w chunk lands
            h1T = miscp.tile([128, 4 * BL], bf16, tag="h1T")
            ps1 = pslin.tile([128, 4 * BL], f32, tag="lin")
            for mt in range(4):
                for kt in range(16):
                    nc.tensor.matmul(
                        ps1[:, mt * BL:(mt + 1) * BL],
                        fc1w_sb[:, kt * 512 + mt * 128: kt * 512 + (mt + 1) * 128],
                        xT_sb[:, kt * BL:(kt + 1) * BL],
                        start=(kt == 0), stop=(kt == 15))
                nc.scalar.activation(
                    h1T[:, mt * BL:(mt + 1) * BL], ps1[:, mt * BL:(mt + 1) * BL],
                    AF.Relu, bias=fc1b_sb[:, mt:mt + 1])

            # ---- mid-priority loads (small; needed for c0/c1 + BN) ----
            gbc_sb = constp.tile([128, 6], f32, tag="gbc")
            nc.sync.dma_start(gbc_sb[:], gbcol[:])
            epsc = constp.tile([128, 1], f32, tag="epsc")
            nc.gpsimd.memset(epsc[:], EPS)
            sT_sb = {64: constp.tile([128, 256], f32, tag="sT64", name="sT64sb"),
                     32: constp.tile([128, 256], f32, tag="sT32", name="sT32sb")}
            nc.sync.dma_start(sT_sb[64][:], selT64[:])
            nc.sync.dma_start(sT_sb[32][:], selT32[:])
            b3_sb = constp.tile([1, 96], bf16, tag="b3row")
            nc.sync.dma_start(b3_sb[:], b3row[:])

            LUT, LTd = {}, {}
            t = wpool.tile([128, 320], bf16, tag="LU0")
            nc.sync.dma_start(t[:], LU0[:])
            LUT["c0"] = t
            t = wpool.tile([128, 3 * 320], bf16, tag="LT1")
            nc.sync.dma_start(t[:], LT1[:])
            LTd["c0"] = LTd["c1"] = LUT["c1"] = t
            wall_sb = wpool.tile([128, WPACK_N], bf16, tag="wall")
            nc.sync.dma_start(wall_sb[:], wall[:])
            W_sb = {}
            for cfg in CFGS:
                for w in "ABC":
                    o, n = _WOFF[f"{w}{cfg.name}"]
                    W_sb[f"{w}{cfg.name}"] = wall_sb[:, o:o + n]

            # ================= fc2 (streamed in 4 column-chunks) =========
            # psum partition = (v0%2)*64+f, col = mi*BL+b ; channels c = v0*64+f.
            # dest: XF0[(b%2)*64+f, (b//2)*80 + v0],  v0 = 2*(mc*10+mi)+p0
            XF0 = poolC.tile([128, 16 * 80], bf16, tag="XF0")
            cfg0 = CFGS[0]
            XFrep0 = poolA.tile([128, cfg0.nG * cfg0.V], bf16, tag="A")
            s_r0 = XF0[:].rearrange("p (g w) -> p g w", w=80)
            d_r0 = XFrep0[:].rearrange("p (g w r) -> p g w r", w=80, r=4)
            for mc in range(4):
                wch = poolB.tile([128, 4 * 1280], bf16, tag="B")
                nc.gpsimd.dma_start(
                    wch[:], fc2wt[:, mc * 4 * 1280:(mc + 1) * 4 * 1280])
                ps2 = psbig.tile([128, 10 * BL], f32, tag="big")
                for mi in range(10):
                    for kt in range(4):
                        nc.tensor.matmul(
                            ps2[:, mi * BL:(mi + 1) * BL],
                            wch[:, kt * 1280 + mi * 128: kt * 1280 + (mi + 1) * 128],
                            h1T[:, kt * BL:(kt + 1) * BL],
                            start=(kt == 0), stop=(kt == 3))
                src4 = ps2[:].rearrange("p (i g j) -> p i g j", g=16, j=2)
                dst4 = XF0[:].rearrange("p (g u q) -> p g u q", u=40, q=2)
                for p0 in range(2):
                    for j in range(2):
                        nc.scalar.activation(
                            dst4[j * 64:(j + 1) * 64, :,
                                 mc * 10:(mc + 1) * 10, p0]
                            .rearrange("p g i -> p i g"),
                            src4[p0 * 64:(p0 + 1) * 64, :, :, j],
                            AF.Copy)
                # up4-replicate this chunk's w-range (w = v0 in [20mc, 20mc+20))
                for r in range(4):
                    nc.vector.tensor_copy(
                        d_r0[:, :, 20 * mc:20 * (mc + 1), r],
                        s_r0[:, :, 20 * mc:20 * (mc + 1)])

            # ---- big late loads (needed at c2; stream during c0/c1) ----
            t = wpool.tile([128, 3 * 1280], fp8, tag="LU2")
            nc.gpsimd.dma_start(t[:], LU2[:])
            LUT["c2"] = t
            t = wpool.tile([128, 10 * 1280], fp8, tag="LT2")
            nc.gpsimd.dma_start(t[:, :5 * 1280], LT2[:, :5 * 1280])
            nc.gpsimd.dma_start(t[:, 5 * 1280:], LT2[:, 5 * 1280:])
            LTd["c2"] = LTd["c3"] = LUT["c3"] = t

            # ================= cheby layers =================
            if USE_RDMA:
                rsem = nc.alloc_semaphore("bn_rsem")
                lsem = nc.alloc_semaphore("bn_lsem")
                rbufs = [constp.tile([128, 16], f32, tag=f"rbuf{i}",
                                     name=f"rbuf{i}")
                         for i in range(3)]
            XF_cur = XF0
            XFrep_cur = XFrep0
            ar_idx = 0

            for li, cfg in enumerate(CFGS):
                V, Vsp, F = cfg.V, cfg.Vsp, cfg.Fout
                BF = cfg.BF
                last = cfg.name == "c3"
                XFrep = XFrep_cur if cfg.up4 else XF_cur

                fp8sp = cfg.name in ("c2", "c3")
                sp_dt = fp8 if fp8sp else bf16
                # --- C linear (in Vsp space) ---
                XC = poolC.tile([128, cfg.nVsp * BL * F], sp_dt, tag="XC")
                gpack = max(1, 512 // cfg.GF)
                for s in range(cfg.nVsp):
                    ssz = cfg.sps(s)
                    for g0 in range(0, cfg.nG, gpack):
                        gn = min(gpack, cfg.nG - g0)
                        pc = pslin.tile([128, 512], f32, tag="lin")
                        for gi in range(gn):
                            g = g0 + gi
                            nc.tensor.matmul(
                                pc[:ssz, gi * cfg.GF:(gi + 1) * cfg.GF],
                                XF_cur[:, g * Vsp + s * 128:
                                       g * Vsp + s * 128 + ssz],
                                W_sb[f"C{cfg.name}"][:],
                                start=True, stop=True)
                        nc.scalar.activation(
                            XC[:ssz, s * BL * F + g0 * cfg.GF:
                               s * BL * F + (g0 + gn) * cfg.GF],
                            pc[:ssz, :gn * cfg.GF], AF.Copy)

                # --- inner = LU @ (2C) + B ;  y = L @ inner + A ---
                Xin = poolB.tile([128, cfg.nVt * BF], sp_dt, tag="B")
                ytile = poolC.tile([128, cfg.nVt * BF], bf16, tag="YT")
                if not last:
                    # transpose/stat state, filled per-tile inside phase 1
                    Gp, nGp = cfg.Gp, cfg.nGp
                    nq = (nGp + 3) // 4
                    XFn = poolA.tile([128, nGp * V], bf16, tag="A")
                    dstv = XFn[:].rearrange("p (g v) -> p g v", v=V)
                    nch = cfg.nVt * nq
                    bnst = miscp.tile([128, nch * 6], f32, tag="bnst")
                    chn = 0
                for phase in range(2):
                    srcL = LUT[cfg.name] if phase == 0 else LTd[cfg.name]
                    nS = cfg.nVsp if phase == 0 else cfg.nVt
                    ssizes = ([cfg.sps(s) for s in range(nS)] if phase == 0
                              else [cfg.vts(s) for s in range(nS)])
                    rhs = XC if phase == 0 else Xin
                    rhs_w = BL * F if phase == 0 else BF
                    Wacc = W_sb[f"B{cfg.name}" if phase == 0 else f"A{cfg.name}"]
                    dst = Xin if phase == 0 else ytile
                    for t in range(cfg.nVt):
                        vsz = cfg.vts(t)
                        for n0 in range(0, BF, 512):
                            n1 = min(n0 + 512, BF)
                            pw = n1 - n0
                            pc0 = n0
                            pi = psbig.tile([128, 512], f32, tag="big")
                            if True:
                                if fp8sp:
                                    # fp8 DoubleRow: contract 2 s-tiles/pass
                                    srcr = srcL[:].rearrange(
                                        "p (s v) -> p s v", v=V)
                                    rhsr = rhs[:].rearrange(
                                        "p (s n) -> p s n", n=rhs_w)
                                    for s0 in range(0, nS, 2):
                                        if s0 + 1 < nS and \
                                                ssizes[s0 + 1] == 128:
                                            nc.tensor.matmul(
                                                pi[:vsz, n0 - pc0:n1 - pc0],
                                                srcr[:, s0:s0 + 2,
                                                     t * 128:t * 128 + vsz],
                                                rhsr[:, s0:s0 + 2, n0:n1],
                                                start=(s0 == 0), stop=False,
                                                skip_group_check=True,
                                                perf_mode=DR)
                                        else:
                                            for s in range(s0, min(s0 + 2,
                                                                   nS)):
                                                ssz = ssizes[s]
                                                nc.tensor.matmul(
                                                    pi[:vsz,
                                                       n0 - pc0:n1 - pc0],
                                                    srcL[:ssz, s * V + t * 128:
                                                         s * V + t * 128 + vsz],
                                                    rhs[:ssz, s * rhs_w + n0:
                                                        s * rhs_w + n1],
                                                    start=(s == 0), stop=False,
                                                    skip_group_check=True)
                                else:
                                    for s in range(nS):
                                        ssz = ssizes[s]
                                        nc.tensor.matmul(
                                            pi[:vsz, n0 - pc0:n1 - pc0],
                                            srcL[:ssz, s * V + t * 128:
                                                 s * V + t * 128 + vsz],
                                            rhs[:ssz, s * rhs_w + n0:
                                                s * rhs_w + n1],
                                            start=(s == 0), stop=False,
                                            skip_group_check=True)
                                for g in range(n0 // cfg.GF,
                                               (n1 + cfg.GF - 1) // cfg.GF):
                                    nc.tensor.matmul(
                                        pi[:vsz, g * cfg.GF - pc0:
                                           (g + 1) * cfg.GF - pc0],
                                        XFrep[:, g * V + t * 128:
                                              g * V + t * 128 + vsz],
                                        Wacc[:],
                                        start=False,
                                        stop=(not (last and phase == 1)),
                                        skip_group_check=True)
                                if last and phase == 1:
                                    # fold cl3 bias: += ones^T @ b3row
                                    nc.tensor.matmul(
                                        pi[:vsz, n0 - pc0:n1 - pc0],
                                        onesv[:1, :vsz],
                                        b3_sb[:1, n0:n1],
                                        start=False, stop=True,
                                        skip_group_check=True)
                            if last and phase == 1:
                                nc.vector.tensor_copy(
                                    dst[:vsz, t * BF + pc0: t * BF + pc0 + pw],
                                    pi[:vsz, :pw])
                            elif phase == 0:
                                if (t * BF + n0) // 512 % 2 == 0:
                                    nc.scalar.activation(
                                        dst[:vsz,
                                            t * BF + pc0: t * BF + pc0 + pw],
                                        pi[:vsz, :pw], AF.Copy)
                                else:
                                    nc.vector.tensor_copy(
                                        dst[:vsz,
                                            t * BF + pc0: t * BF + pc0 + pw],
                                        pi[:vsz, :pw])
                            else:
                                if (t * BF + n0) // 512 % 2 == 0:
                                    nc.vector.tensor_copy(
                                        dst[:vsz,
                                            t * BF + pc0: t * BF + pc0 + pw],
                                        pi[:vsz, :pw])
                                else:
                                    nc.scalar.activation(
                                        dst[:vsz,
                                            t * BF + pc0: t * BF + pc0 + pw],
                                        pi[:vsz, :pw], AF.Copy)
                        if phase == 1 and not last:
                            # back-transpose this tile now so PE overlaps
                            # later tiles' spmm matmuls
                            for qi0 in range(nq):
                                q0 = qi0 * 4
                                qn = min(4, nGp - q0)
                                pt = pstr.tile([128, 512], bf16, tag="tr")
                                for qi in range(qn):
                                    gp = q0 + qi
                                    nc.tensor.transpose(
                                        pt[:, qi * vsz: qi * vsz + vsz],
                                        ytile[:vsz, t * BF + gp * 128:
                                              t * BF + (gp + 1) * 128],
                                        ident_b[:vsz, :vsz])
                                reg = dstv[:, q0:q0 + qn,
                                           t * 128:t * 128 + vsz]
                                nc.scalar.activation(
                                    reg,
                                    pt[:].rearrange("p (q v) -> p q v",
                                                    v=vsz)[:, :qn, :],
                                    AF.Copy)
                                nc.vector.bn_stats(
                                    bnst[:, chn * 6:(chn + 1) * 6],
                                    pt[:, :qn * vsz])
                                chn += 1

                if not last:
                    aggr = miscp.tile([128, 2], f32, tag="aggr")
                    nc.vector.bn_aggr(
                        aggr[:], bnst[:, :chn * 6]
                        .rearrange("p (c s) -> p c s", s=6))
                    part = miscp.tile([128, 2], f32, tag="part")
                    if USE_RDMA and ar_idx > 0:
                        # prior layer's sends must have drained before reuse
                        nc.vector.wait_ge(lsem, 112 * ar_idx)
                    nc.vector.tensor_tensor(
                        out=part[:, 1:2], in0=aggr[:, 0:1], in1=aggr[:, 0:1],
                        op=ALU.mult)
                    nc.vector.tensor_tensor(
                        out=part[:, 1:2], in0=part[:, 1:2], in1=aggr[:, 1:2],
                        op=ALU.add)
                    if USE_RDMA:
                        nc.vector.tensor_copy(part[:, 0:1], aggr[:, 0:1])
                        rb = rbufs[ar_idx]
                        nc.vector.tensor_copy(rb[:, 0:2], part[:])
                        for k in range(1, 8):
                            nc.gpsimd.remote_dma_broadcast(
                                rb[:, 2 * k:2 * k + 2], part[:],
                                remote_sem=rsem, local_sem=lsem,
                                rdests=[(0, k) if i == k else None
                                        for i in range(8)])
                        nc.gpsimd.trigger_dma(count=None)
                        nc.vector.wait_ge(rsem, 14 * (ar_idx + 1))
                        nc.vector.tensor_tensor(
                            out=rb[:, 0:8], in0=rb[:, 0:8], in1=rb[:, 8:16],
                            op=ALU.add)
                        nc.vector.tensor_tensor(
                            out=rb[:, 0:4], in0=rb[:, 0:4], in1=rb[:, 4:8],
                            op=ALU.add)
                        nc.vector.tensor_tensor(
                            out=rb[:, 0:2], in0=rb[:, 0:2], in1=rb[:, 2:4],
                            op=ALU.add)
                        pst = pslin.tile([128, 512], f32, tag="lin")
                        nc.tensor.matmul(pst[:1, :F], rb[:, 0:1],
                                         selfd_sb[li], start=True, stop=True)
                        nc.tensor.matmul(pst[:1, F:2 * F], rb[:, 1:2],
                                         selfd_sb[li], start=True, stop=True)
                        stats_g = miscp.tile([1, 2 * F], f32, tag="statg")
                        nc.vector.tensor_copy(stats_g[:], pst[:1, :2 * F])
                    else:
                        pst = pslin.tile([128, 512], f32, tag="lin")
                        nc.tensor.matmul(pst[:1, :F], aggr[:, 0:1],
                                         selfd_sb[li], start=True, stop=True)
                        nc.tensor.matmul(pst[:1, F:2 * F], part[:, 1:2],
                                         selfd_sb[li], start=True, stop=True)
                        stats_l = miscp.tile([1, 2 * F], f32, tag="statl")
                        nc.vector.tensor_copy(stats_l[:], pst[:1, :2 * F])
                        bin_ = dramp.tile([1, 2 * F], f32, tag=f"arin{ar_idx}")
                        bout = dramp.tile([8, 2 * F], f32, tag=f"arout{ar_idx}")
                        nc.sync.dma_start(bin_[:], stats_l[:])
                        nc.gpsimd.collective_compute(
                            "AllGather", ALU.bypass,
                            replica_groups=[list(range(NCORES))],
                            ins=[bin_.opt()], outs=[bout.opt()])
                        sg8 = miscp.tile([8, 2 * F], f32, tag="sg8")
                        nc.sync.dma_start(sg8[:], bout[:])
                        psg = pslin.tile([128, 512], f32, tag="lin", name="psg")
                        nc.tensor.matmul(psg[:2 * F, 0:1], sg8[:, :2 * F],
                                         onesn[:, li:li + 1],
                                         start=True, stop=True)
                    # column form, all vars at partitions [0:F]
                    # stg col0 rows [0:F]=mu, [F:2F]=E[y^2]
                    stg = miscp.tile([128, 1], f32, tag="stg")
                    nc.vector.tensor_copy(stg[:2 * F, :], psg[:2 * F, 0:1])
                    w = miscp.tile([128, 2], f32, tag="bnw")
                    st = miscp.tile([128, 2], f32, tag="st")
                    # w1 = mu*mu - E[y^2] = -var  (scalar2 reads rows F:2F)
                    nc.vector.tensor_scalar(
                        out=w[0:F, 1:2], in0=stg[0:F, :],
                        scalar1=stg[0:F, :], scalar2=stg[F:2 * F, :],
                        op0=ALU.mult, op1=ALU.subtract)
                    nc.scalar.activation(w[0:F, 1:2], w[0:F, 1:2],
                                         AF.Sqrt, bias=epsc[0:F, :],
                                         scale=-1.0)
                    nc.vector.reciprocal(w[0:F, 1:2], w[0:F, 1:2])
                    nc.vector.tensor_tensor(out=st[0:F, 0:1],
                                            in0=w[0:F, 1:2],
                                            in1=gbc_sb[0:F, 2 * li:2 * li + 1],
                                            op=ALU.mult)
                    # st1 = mu*s - beta = -t  (negated by the t-map below)
                    nc.vector.tensor_scalar(
                        out=st[0:F, 1:2], in0=stg[0:F, :],
                        scalar1=st[0:F, 0:1],
                        scalar2=gbc_sb[0:F, 2 * li + 1:2 * li + 2],
                        op0=ALU.mult, op1=ALU.subtract)
                    # broadcast: col0 via +map, col1 via -map (restores t)
                    psc = pslin.tile([128, 512], f32, tag="lin", name="psc")
                    nc.tensor.matmul(psc[:, 0:1], sT_sb[F][:F, 0:128],
                                     st[:F, 0:1], start=True, stop=True)
                    nc.tensor.matmul(psc[:, 1:2], sT_sb[F][:F, 128:256],
                                     st[:F, 1:2], start=True, stop=True)
                    stc = miscp.tile([128, 2], f32, tag=f"stc{ar_idx}")
                    nc.vector.tensor_copy(stc[:], psc[:, 0:2])
                    ar_idx += 1
                    # chunked scale+relu (+ up4 replication for next layer)
                    ncfg = CFGS[li + 1]
                    if ncfg.up4:
                        XFrep_cur = poolA.tile(
                            [128, ncfg.nG * ncfg.V], bf16, tag="A")
                        s_r = XFn[:].rearrange("p (g w) -> p g w", w=V)
                        d_r = XFrep_cur[:].rearrange(
                            "p (g w r) -> p g w r", w=V, r=4)
                    nap = max(1, nGp // 4)
                    bnds = [0, 1] + list(range(1 + nap, nGp, nap)) + [nGp]
                    bnds = sorted(set(b for b in bnds if b <= nGp))
                    for q0, q1 in zip(bnds[:-1], bnds[1:]):
                        c0_, c1_ = q0 * V, q1 * V
                        cm = c0_ + (c1_ - c0_) * 5 // 9  # Act a bit slower
                        nc.scalar.activation(
                            XFn[:, c0_:cm], XFn[:, c0_:cm],
                            AF.Relu, scale=stc[:, 0:1], bias=stc[:, 1:2])
                        nc.vector.tensor_scalar(
                            out=XFn[:, cm:c1_], in0=XFn[:, cm:c1_],
                            scalar1=stc[:, 0:1], scalar2=stc[:, 1:2],
                            op0=ALU.mult, op1=ALU.add)
                        nc.vector.tensor_scalar_max(
                            XFn[:, cm:c1_], XFn[:, cm:c1_], 0.0)
                        if ncfg.up4:
                            for r in range(4):
                                nc.vector.tensor_copy(
                                    d_r[:, q0:q1, :, r], s_r[:, q0:q1, :])
                    XF_cur = XFn
                else:
                    # --- output: ship ytile [v-part, (b,c)] as-is; host
                    # untangles the (p, t, b, c) layout in numpy ---
                    for t0, t1 in ((0, 4), (4, 8), (8, 10)):
                        nc.sync.dma_start(
                            ydram[:, t0 * BF:t1 * BF],
                            ytile[:, t0 * BF:t1 * BF])

    nc.compile()
    return nc


def kernel(**inputs):
    import sys
    for p in ("/opt/trn_rl_repo", "/opt/trn_rl_repo/concourse"):
        if p not in sys.path:
            sys.path.insert(0, p)
    from concourse.bass_utils import run_bass_kernel_spmd

    host = _build_host(inputs)

    if "nc" not in _CACHE:
        _CACHE["nc"] = _build_nc()
    nc = _CACHE["nc"]

    in_maps = []
    for c in range(NCORES):
        m = {k: v for k, v in host.items() if k != "xT"}
        m["xT"] = np.ascontiguousarray(host["xT"][:, c * BL:(c + 1) * BL])
        in_maps.append(m)
    res = run_bass_kernel_spmd(nc, in_maps, core_ids=list(range(NCORES)))
    # y[p, t*96 + b*3 + c] -> out[b, t*128+p, c]
    outs = []
    for r in res.results:
        y = np.asarray(r["y"], np.float32).reshape(128, 10, BL, 3)
        outs.append(y.transpose(2, 1, 0, 3).reshape(BL, 1280, 3))
    return np.concatenate(outs, axis=0).astype(np.float32)


if __name__ == "__main__":
    import reference as R
    inp = R.setup_inputs()
    inp = {k: np.asarray(v) for k, v in inp.items()}
    act = kernel(**inp)
    exp = np.asarray(R.reference(**inp))
    err = np.linalg.norm(act - exp) / np.linalg.norm(exp)
    print("Relative error:", err)
